# revision 1
# baseline (speedup 1.0000x reference)
"""Trainium2 Bass kernel for the 19-class mean-IoU (DiceLoss) problem.

Full-input contract: kernel(input, target) takes the FULL unsharded inputs
(input [4,19,512,1024] f32, target [4,512,1024] int), returns the scalar
f32 mean-IoU.  Internally the 2,097,152 pixels are sharded across 8
NeuronCores (data-parallel over the flattened batch*H*W pixel axis); each
core computes per-class partial counts (pred / label / intersection) for
its 262,144 pixels, the 8 count vectors are summed on host and the final
division happens on host (all tiny: 8 x 57 floats).

Per-core device algorithm (per block of 128x512 pixels):
  - DMA a class-major tile X[p=128, c=19, f=512] (2KB contiguous runs)
  - m = reduce_max over classes (DVE, strided innermost AP)
  - per class c: eq_c = (x_c == m) with fused accum -> pred counts
                 (t==c)*eq_c  with fused accum -> intersection counts
                 (t==c)       with fused accum -> label counts
  - final: per-block accums reduced, then a ones-vector matmul on the PE
    collapses the partition axis; one 57-float DMA out per core.
"""

import numpy as np

import concourse.bass as bass
import concourse.mybir as mybir
from concourse import bass_utils
from concourse.tile import TileContext
from concourse.tile_rust import add_dep_helper

C = 19          # classes
P = 128         # SBUF partitions
NCORES = 8
F = 1024        # pixels per partition per block

_Alu = mybir.AluOpType
_Ax = mybir.AxisListType
_dt = mybir.dt

VERSION = 4          # 1: all-DVE; 2: groups; 3: GP-max; 4: DVE+ACT minimal
NS = 21              # label |t-c| accumulator count (c = -1..19)
OUTN = 19 + NS + 19  # pred counts, S values, inter counts


def _body_v4(tc, x, t, out, n, f):
    """Minimal-risk engine split (all probe-verified ops): DVE does the max
    reduce and per-class eq->inter (fused accum STT, one shared EQ tile),
    ACT does the label S-histogram.  X blocks stay resident so DMAs carry
    no waits; manual pre-drains keep the tail drain within the 1-sync-wait
    walrus limit."""
    nc = tc.nc
    nb = n // (P * f)
    bf16 = _dt.bfloat16
    with tc.tile_pool(name="xp", bufs=nb) as xp, \
         tc.tile_pool(name="tp", bufs=nb) as tp, \
         tc.tile_pool(name="mp", bufs=1) as mp, \
         tc.tile_pool(name="eqp", bufs=1) as eqp, \
         tc.tile_pool(name="lsp", bufs=1) as lsp, \
         tc.tile_pool(name="accp", bufs=1) as accp, \
         tc.tile_pool(name="psp", bufs=1, space="PSUM") as psp:
        pacc = accp.tile([P, nb * C], _dt.float32, tag="pacc")
        lacc = accp.tile([P, nb * NS], _dt.float32, tag="lacc")
        iacc = accp.tile([P, nb * C], _dt.float32, tag="iacc")
        ones = accp.tile([P, 1], _dt.float32, tag="ones")
        nc.vector.memset(ones[:, :], 1.0)
        biases = accp.tile([P, NS], _dt.float32, tag="biases")
        for j in range(NS):
            nc.vector.memset(biases[:, j:j + 1], float(1 - j))
        ashim = accp.tile([P, nb], _dt.float32, tag="ashim")
        xshim = accp.tile([P, nb], _dt.float32, tag="xshim")

        pre_drain_hooks = []
        T8 = accp.tile([P, n // P], _dt.uint8, tag="T8")
        t8d = nc.sync.dma_start(
            T8[:, :].rearrange("p (b f) -> p b f", f=f),
            t.rearrange("(b p f) -> p b f", p=P, f=f),
        )
        pre_drain_hooks.append(t8d)
        xs = []
        for b in range(nb):
            X = xp.tile([P, C * f], _dt.float32, tag="X")
            xs.append(X)
            xd = nc.sync.dma_start(
                X[:, :].rearrange("p (c f) -> p c f", c=C),
                x[:, b * P * f:(b + 1) * P * f].rearrange(
                    "c (p f) -> p c f", p=P),
            )
            pre_drain_hooks.append(xd)

        small_dve = []
        last_ab = None
        for b in range(nb):
            X = xs[b]
            Tf = tp.tile([P, f], _dt.float32, tag="Tf")
            cast = nc.vector.tensor_copy(Tf[:, :], T8[:, b * f:(b + 1) * f])
            small_dve.append(cast)
            # ACT: entry shim + 21 |t-c| accumulations
            ash = nc.scalar.activation(
                ashim[:, b:b + 1], Tf[:, 0:1],
                mybir.ActivationFunctionType.Copy,
            )
            if last_ab is not None:
                add_dep_helper(ash.ins, last_ab.ins, sync=False,
                               reason="chain ACT blocks")
            LS = lsp.tile([P, f], _dt.float32, tag="LS")
            for j in range(NS):
                ab = nc.scalar.activation(
                    LS[:, :], Tf[:, :], mybir.ActivationFunctionType.Abs,
                    bias=biases[:, j:j + 1], scale=1.0,
                    accum_out=lacc[:, b * NS + j: b * NS + j + 1],
                )
                add_dep_helper(ab.ins, ash.ins, sync=False,
                               reason="keep ACT shim first")
            last_ab = ab

            # DVE: X shim absorbs the DMA wait, then max reduce, then per
            # class: eq mask (+pred count) and intersection count
            xsh = nc.vector.tensor_copy(xshim[:, b:b + 1], X[:, 0:1])
            small_dve.append(xsh)
            M = mp.tile([P, f], _dt.float32, tag="M")
            red = nc.vector.tensor_tensor(
                M[:, :], X[:, 0:f], X[:, f:2 * f], _Alu.max
            )
            add_dep_helper(red.ins, xsh.ins, sync=False,
                           reason="keep DVE X shim first")
            for c in range(2, C):
                nc.vector.tensor_tensor(
                    M[:, :], M[:, :], X[:, c * f:(c + 1) * f], _Alu.max
                )
            EQ1 = eqp.tile([P, f], _dt.float32, tag="EQ1")
            for c in range(C):
                nc.vector.scalar_tensor_tensor(
                    out=EQ1[:, :],
                    in0=X[:, c * f:(c + 1) * f],
                    scalar=0.0,
                    in1=M[:, :],
                    op0=_Alu.bypass,
                    op1=_Alu.is_equal,
                    accum_out=pacc[:, b * C + c: b * C + c + 1],
                )
                nc.vector.scalar_tensor_tensor(
                    out=EQ1[:, :],
                    in0=Tf[:, :],
                    scalar=float(c),
                    in1=EQ1[:, :],
                    op0=_Alu.is_equal,
                    op1=_Alu.mult,
                    accum_out=iacc[:, b * C + c: b * C + c + 1],
                )

        CNT = accp.tile([P, OUTN], _dt.float32, tag="CNT")
        nc.vector.tensor_reduce(
            CNT[:, 0:C],
            pacc[:, :].rearrange("p (b c) -> p c b", c=C),
            axis=_Ax.X, op=_Alu.add,
        )
        nc.vector.tensor_reduce(
            CNT[:, C:C + NS],
            lacc[:, :].rearrange("p (b c) -> p c b", c=NS),
            axis=_Ax.X, op=_Alu.add,
        )
        nc.vector.tensor_reduce(
            CNT[:, C + NS:OUTN],
            iacc[:, :].rearrange("p (b c) -> p c b", c=C),
            axis=_Ax.X, op=_Alu.add,
        )
        PS = psp.tile([1, OUTN], _dt.float32, tag="PS")
        mm = nc.tensor.matmul(
            PS[:, :], ones[:, :], CNT[:, :], start=True, stop=True
        )
        OUT = accp.tile([1, OUTN], _dt.float32, tag="OUT")
        oc = nc.vector.tensor_copy(OUT[:, :], PS[:, :])
        for s in small_dve:
            add_dep_helper(oc.ins, s.ins, sync=False,
                           reason="OUT copy last on DVE")
        od = nc.sync.dma_start(out.rearrange("(o k) -> o k", o=1), OUT[:, :])

        pre_drain_hooks += [last_ab, mm, od]
        for h in pre_drain_hooks:
            dr = nc.sync.drain()
            add_dep_helper(dr.ins, h.ins, sync=True, reason="pre-drain")


def _body_v3(tc, x, t, out, n, f):
    """v3 engine split: GPSIMD computes the class max (18 plain TT max ops,
    ping-pong), DVE does the eq + intersection passes (fused accum ops,
    which only DVE supports), ACT does the label S-histogram.  Every data
    instruction carries at most one sync wait (walrus limit):
    - all X blocks resident -> X DMAs carry no waits
    - GP enters a block via a tiny copy shim whose DVE wait covers the
      MR-slot readers of two blocks ago (via the eqmark marker column)
    - DVE enters via an X shim (absorbs the DMA wait); the first eq op
      then only waits on Pool (the GP max result)
    """
    nc = tc.nc
    nb = n // (P * f)
    bf16 = _dt.bfloat16
    with tc.tile_pool(name="xp", bufs=nb) as xp, \
         tc.tile_pool(name="tp", bufs=nb) as tp, \
         tc.tile_pool(name="mrp", bufs=4) as mrp, \
         tc.tile_pool(name="eqp", bufs=1) as eqp, \
         tc.tile_pool(name="lsp", bufs=1) as lsp, \
         tc.tile_pool(name="scp", bufs=2) as scp, \
         tc.tile_pool(name="accp", bufs=1) as accp, \
         tc.tile_pool(name="psp", bufs=1, space="PSUM") as psp:
        pacc = accp.tile([P, nb * C], _dt.float32, tag="pacc")
        lacc = accp.tile([P, nb * NS], _dt.float32, tag="lacc")
        iacc = accp.tile([P, nb * C], _dt.float32, tag="iacc")
        ones = accp.tile([P, 1], _dt.float32, tag="ones")
        nc.vector.memset(ones[:, :], 1.0)
        biases = accp.tile([P, NS], _dt.float32, tag="biases")
        for j in range(NS):
            nc.vector.memset(biases[:, j:j + 1], float(1 - j))
        ashim = accp.tile([P, nb], _dt.float32, tag="ashim")
        xshim = accp.tile([P, nb], _dt.float32, tag="xshim")
        eqmark = accp.tile([P, nb], bf16, tag="eqmark")
        gshim = accp.tile([P, nb * 32], bf16, tag="gshim")

        pre_drain_hooks = []
        T8 = accp.tile([P, n // P], _dt.uint8, tag="T8")
        t8d = nc.sync.dma_start(
            T8[:, :].rearrange("p (b f) -> p b f", f=f),
            t.rearrange("(b p f) -> p b f", p=P, f=f),
        )
        pre_drain_hooks.append(t8d)
        xs = []
        for b in range(nb):
            X = xp.tile([P, C * f], _dt.float32, tag="X")
            xs.append(X)
            xd = nc.sync.dma_start(
                X[:, :].rearrange("p (c f) -> p c f", c=C),
                x[:, b * P * f:(b + 1) * P * f].rearrange(
                    "c (p f) -> p c f", p=P),
            )
            pre_drain_hooks.append(xd)

        small_dve = []
        last_ab = None
        last_gp = None
        for b in range(nb):
            X = xs[b]
            # target cast on DVE (uint8 -> bf16; values 0..18 exact)
            Tf = tp.tile([P, f], bf16, tag="Tf")
            cast = nc.vector.tensor_copy(Tf[:, :], T8[:, b * f:(b + 1) * f])
            small_dve.append(cast)
            # ACT entry shim + label S-histogram
            ash = nc.scalar.activation(
                ashim[:, b:b + 1], Tf[:, 0:1],
                mybir.ActivationFunctionType.Copy,
            )
            if last_ab is not None:
                add_dep_helper(ash.ins, last_ab.ins, sync=False,
                               reason="chain ACT blocks")
            LS = lsp.tile([P, f], bf16, tag="LS")
            for j in range(NS):
                ab = nc.scalar.activation(
                    LS[:, :], Tf[:, :], mybir.ActivationFunctionType.Abs,
                    bias=biases[:, j:j + 1], scale=1.0,
                    accum_out=lacc[:, b * NS + j: b * NS + j + 1],
                )
                add_dep_helper(ab.ins, ash.ins, sync=False,
                               reason="keep ACT shim first")
            last_ab = ab

            # GPSIMD: running max over the 19 classes (ping-pong buffers).
            # Entry shim: reading eqmark(b-2) folds the MR-slot reader
            # dependency into one DVE wait without stalling behind newer
            # DVE work; the first max op carries the X-DMA wait.
            gsh = None
            if b >= 2:
                gsh = nc.gpsimd.tensor_copy(
                    gshim[:, b * 32:(b + 1) * 32],
                    eqmark[:, b - 2:b - 1].to_broadcast([P, 32]),
                )
                if last_gp is not None:
                    add_dep_helper(gsh.ins, last_gp.ins, sync=False,
                                   reason="chain GP blocks")
            MRa = mrp.tile([P, f], _dt.float32, tag="MR")
            MRb = mrp.tile([P, f], _dt.float32, tag="MR")
            mr = [MRa, MRb]
            g0 = nc.gpsimd.tensor_tensor(
                MRa[:, :], X[:, 0:f], X[:, f:2 * f], _Alu.max
            )
            if gsh is not None:
                add_dep_helper(g0.ins, gsh.ins, sync=False,
                               reason="keep GP shim first")
            elif last_gp is not None:
                add_dep_helper(g0.ins, last_gp.ins, sync=False,
                               reason="chain GP blocks")
            last_gp = g0
            for c in range(2, C):
                gi = nc.gpsimd.tensor_tensor(
                    mr[c % 2][:, :],
                    mr[(c - 1) % 2][:, :],
                    X[:, c * f:(c + 1) * f],
                    _Alu.max,
                )
                last_gp = gi
            M = mr[(C - 1) % 2]

            # DVE X-entry shim absorbs the X DMA wait
            xsh = nc.vector.tensor_copy(xshim[:, b:b + 1], X[:, 0:1])
            small_dve.append(xsh)
            # DVE: eq masks (+pred counts) then intersection counts
            EQ = eqp.tile([P, C * f], bf16, tag="EQ")
            for c in range(C):
                eqi = nc.vector.scalar_tensor_tensor(
                    out=EQ[:, c * f:(c + 1) * f],
                    in0=X[:, c * f:(c + 1) * f],
                    scalar=0.0,
                    in1=M[:, :],
                    op0=_Alu.bypass,
                    op1=_Alu.is_equal,
                    accum_out=pacc[:, b * C + c: b * C + c + 1],
                )
                add_dep_helper(eqi.ins, xsh.ins, sync=False,
                               reason="keep DVE X shim first")
            # generation marker: tick provably after this block's eq ops
            em = nc.vector.tensor_copy(
                eqmark[:, b:b + 1], EQ[:, C * f - 1:C * f]
            )
            small_dve.append(em)
            GS = scp.tile([P, f], bf16, tag="GS")
            for c in range(C):
                nc.vector.scalar_tensor_tensor(
                    out=GS[:, :],
                    in0=Tf[:, :],
                    scalar=float(c),
                    in1=EQ[:, c * f:(c + 1) * f],
                    op0=_Alu.is_equal,
                    op1=_Alu.mult,
                    accum_out=iacc[:, b * C + c: b * C + c + 1],
                )

        CNT = accp.tile([P, OUTN], _dt.float32, tag="CNT")
        nc.vector.tensor_reduce(
            CNT[:, 0:C],
            pacc[:, :].rearrange("p (b c) -> p c b", c=C),
            axis=_Ax.X, op=_Alu.add,
        )
        nc.vector.tensor_reduce(
            CNT[:, C:C + NS],
            lacc[:, :].rearrange("p (b c) -> p c b", c=NS),
            axis=_Ax.X, op=_Alu.add,
        )
        nc.vector.tensor_reduce(
            CNT[:, C + NS:OUTN],
            iacc[:, :].rearrange("p (b c) -> p c b", c=C),
            axis=_Ax.X, op=_Alu.add,
        )
        PS = psp.tile([1, OUTN], _dt.float32, tag="PS")
        mm = nc.tensor.matmul(
            PS[:, :], ones[:, :], CNT[:, :], start=True, stop=True
        )
        OUT = accp.tile([1, OUTN], _dt.float32, tag="OUT")
        oc = nc.vector.tensor_copy(OUT[:, :], PS[:, :])
        for s in small_dve:
            add_dep_helper(oc.ins, s.ins, sync=False,
                           reason="OUT copy last on DVE")
        od = nc.sync.dma_start(out.rearrange("(o k) -> o k", o=1), OUT[:, :])

        pre_drain_hooks += [last_gp, last_ab, mm, od]
        for h in pre_drain_hooks:
            dr = nc.sync.drain()
            add_dep_helper(dr.ins, h.ins, sync=True, reason="pre-drain")


def _body_v2(tc, x, t, out, n, f):
    """Engine-split version: DVE does max+eq, GPSIMD does intersection,
    ACT does the label S-histogram (second differences of S(c)=sum|t-c|
    recover exact integer counts).

    Walrus accepts at most ONE attached sync wait per data instruction, so
    the structure keeps every instruction at <=1 cross-engine dependency:
    - all X blocks stay resident (no DMA slot reuse -> DMAs carry no waits)
    - eq masks are produced in 4-class group tiles so DVE and GP pipeline
      at group granularity with 2 buffers (and everything fits in SBUF)
    - each engine enters a block/group through a tiny shim op that absorbs
      the whole DVE dependency in one wait; later ops only carry their
      own-engine scratch WAW wait
    """
    nc = tc.nc
    nb = n // (P * f)
    gw = 4                      # classes per eq group
    groups = [(c0, min(c0 + gw, C)) for c0 in range(0, C, gw)]
    ng = len(groups)
    bf16 = _dt.bfloat16
    with tc.tile_pool(name="xp", bufs=nb) as xp, \
         tc.tile_pool(name="tp", bufs=nb) as tp, \
         tc.tile_pool(name="mp", bufs=1) as mp, \
         tc.tile_pool(name="eqp", bufs=2) as eqp, \
         tc.tile_pool(name="lsp", bufs=1) as lsp, \
         tc.tile_pool(name="gsp", bufs=2) as gsp, \
         tc.tile_pool(name="accp", bufs=1) as accp, \
         tc.tile_pool(name="psp", bufs=1, space="PSUM") as psp:
        pacc = accp.tile([P, nb * C], _dt.float32, tag="pacc")
        lacc = accp.tile([P, nb * NS], _dt.float32, tag="lacc")
        iacc = accp.tile([P, nb * C], _dt.float32, tag="iacc")
        ones = accp.tile([P, 1], _dt.float32, tag="ones")
        nc.vector.memset(ones[:, :], 1.0)
        # bias constants 1-j for the ACT Abs ops; built on DVE like every
        # other ACT input so ACT ops wait on a single engine
        biases = accp.tile([P, NS], _dt.float32, tag="biases")
        for j in range(NS):
            nc.vector.memset(biases[:, j:j + 1], float(1 - j))
        ashim = accp.tile([P, nb], _dt.float32, tag="ashim")
        dshim = accp.tile([P, ng * nb], _dt.float32, tag="dshim")
        xshim = accp.tile([P, nb], _dt.float32, tag="xshim")
        gshim = accp.tile([P, ng * nb * 32], _dt.float32, tag="gshim")

        # whole per-core target, loaded once (uint8: values 0..18)
        pre_drain_hooks = []
        T8 = accp.tile([P, n // P], _dt.uint8, tag="T8")
        t8d = nc.sync.dma_start(
            T8[:, :].rearrange("p (b f) -> p b f", f=f),
            t.rearrange("(b p f) -> p b f", p=P, f=f),
        )
        pre_drain_hooks.append(t8d)
        # all X blocks resident: DMAs prefetch back-to-back with no waits
        xs = []
        for b in range(nb):
            X = xp.tile([P, C * f], _dt.float32, tag="X")
            xs.append(X)
            xd = nc.sync.dma_start(
                X[:, :].rearrange("p (c f) -> p c f", c=C),
                x[:, b * P * f:(b + 1) * P * f].rearrange(
                    "c (p f) -> p c f", p=P),
            )
            pre_drain_hooks.append(xd)

        gs_tiles = {}
        small_dve = []
        last_ab = None
        last_gp = None
        for b in range(nb):
            X = xs[b]
            # target cast on DVE (uint8 -> f32)
            Tf = tp.tile([P, f], _dt.float32, tag="Tf")
            cast = nc.vector.tensor_copy(Tf[:, :], T8[:, b * f:(b + 1) * f])
            small_dve.append(cast)
            # ACT entry shim absorbs the Tf dependency; the Abs ops then
            # only carry their own-engine LS WAW wait
            ash = nc.scalar.activation(
                ashim[:, b:b + 1], Tf[:, 0:1],
                mybir.ActivationFunctionType.Copy,
            )
            if last_ab is not None:
                # chain blocks' ACT sections so the last traced Abs is
                # provably the last-scheduled ACT op
                add_dep_helper(ash.ins, last_ab.ins, sync=False,
                               reason="chain ACT blocks")
            LS = lsp.tile([P, f], bf16, tag="LS")
            for j in range(NS):
                ab = nc.scalar.activation(
                    LS[:, :], Tf[:, :], mybir.ActivationFunctionType.Abs,
                    bias=biases[:, j:j + 1], scale=1.0,
                    accum_out=lacc[:, b * NS + j: b * NS + j + 1],
                )
                add_dep_helper(ab.ins, ash.ins, sync=False,
                               reason="keep ACT shim first")
            last_ab = ab

            # DVE X-entry shim absorbs the X DMA wait so the reduce only
            # carries its own-engine M WAW wait
            small_dve.append(
                nc.vector.tensor_copy(xshim[:, b:b + 1], X[:, 0:1])
            )
            # DVE: max over classes, then eq masks per class group
            M = mp.tile([P, f], _dt.float32, tag="M")
            red = nc.vector.tensor_reduce(
                M[:, :],
                X[:, :].rearrange("p (c f) -> p f c", c=C),
                axis=_Ax.X,
                op=_Alu.max,
            )
            # cast before reduce in the DVE stream: the GP shim reads only
            # the last EQ slice and relies on tick(cast) < tick(eq ops)
            add_dep_helper(red.ins, cast.ins, sync=False,
                           reason="cast before reduce")
            for gi, (c0, c1) in enumerate(groups):
                ncg = c1 - c0
                gidx = b * ng + gi
                if gidx >= 2:
                    # DVE-side GP sync shim: reading GS of the group whose
                    # EQ slot this group reuses folds the EQ-slot WAR (GP
                    # readers) into this op's single wait
                    dsh = nc.vector.tensor_copy(
                        dshim[:, gidx:gidx + 1],
                        gs_tiles[gidx - 2][:, 0:1],
                    )
                    small_dve.append(dsh)
                else:
                    dsh = None
                EQ = eqp.tile([P, gw * f], _dt.float32, tag="EQ")
                for i, c in enumerate(range(c0, c1)):
                    eqi = nc.vector.scalar_tensor_tensor(
                        out=EQ[:, i * f:(i + 1) * f],
                        in0=X[:, c * f:(c + 1) * f],
                        scalar=0.0,
                        in1=M[:, :],
                        op0=_Alu.bypass,
                        op1=_Alu.is_equal,
                        accum_out=pacc[:, b * C + c: b * C + c + 1],
                    )
                    if dsh is not None:
                        add_dep_helper(eqi.ins, dsh.ins, sync=False,
                                       reason="keep DVE GP-sync shim first")
                # GP entry shim: copying the last EQ columns makes GP
                # observe the DVE clock past every producer it needs (the
                # cast-before-reduce edge puts Tf below that tick); the
                # STT ops then only carry their own-engine GS WAW wait
                gsh = nc.gpsimd.tensor_copy(
                    gshim[:, gidx * 32:(gidx + 1) * 32],
                    EQ[:, ncg * f - 32:ncg * f],
                )
                if last_gp is not None:
                    # chain GP groups so the last traced STT is provably
                    # the last-scheduled GP op
                    add_dep_helper(gsh.ins, last_gp.ins, sync=False,
                                   reason="chain GP groups")
                GS = gsp.tile([P, f], _dt.float32, tag="GS")
                gs_tiles[gidx] = GS
                for i, c in enumerate(range(c0, c1)):
                    sti = nc.gpsimd.scalar_tensor_tensor(
                        out=GS[:, :],
                        in0=Tf[:, :],
                        scalar=float(c),
                        in1=EQ[:, i * f:(i + 1) * f],
                        op0=_Alu.is_equal,
                        op1=_Alu.mult,
                        accum_out=iacc[:, b * C + c: b * C + c + 1],
                    )
                    add_dep_helper(sti.ins, gsh.ins, sync=False,
                                   reason="keep GP shim first")
                    last_gp = sti

        CNT = accp.tile([P, OUTN], _dt.float32, tag="CNT")
        nc.vector.tensor_reduce(
            CNT[:, 0:C],
            pacc[:, :].rearrange("p (b c) -> p c b", c=C),
            axis=_Ax.X, op=_Alu.add,
        )
        nc.vector.tensor_reduce(
            CNT[:, C:C + NS],
            lacc[:, :].rearrange("p (b c) -> p c b", c=NS),
            axis=_Ax.X, op=_Alu.add,
        )
        nc.vector.tensor_reduce(
            CNT[:, C + NS:OUTN],
            iacc[:, :].rearrange("p (b c) -> p c b", c=C),
            axis=_Ax.X, op=_Alu.add,
        )
        PS = psp.tile([1, OUTN], _dt.float32, tag="PS")
        mm = nc.tensor.matmul(
            PS[:, :], ones[:, :], CNT[:, :], start=True, stop=True
        )
        OUT = accp.tile([1, OUTN], _dt.float32, tag="OUT")
        oc = nc.vector.tensor_copy(OUT[:, :], PS[:, :])
        # pin the stray [P,1] DVE shims before the OUT copy so the OUT copy
        # is the last-scheduled DVE op (its tick covers the whole engine)
        for s in small_dve:
            add_dep_helper(oc.ins, s.ins, sync=False,
                           reason="OUT copy last on DVE")
        od = nc.sync.dma_start(out.rearrange("(o k) -> o k", o=1), OUT[:, :])

        # Pre-drains: the kernel-tail drain waits on every engine and every
        # in-flight DMA lane, overflowing the 1-sync-wait ISA budget.  These
        # manual SP drains (1 wait each) make SP observe all those
        # semaphores first, so Tile elides them from the tail drain.
        pre_drain_hooks += [last_gp, last_ab, mm, od]
        for h in pre_drain_hooks:
            dr = nc.sync.drain()
            add_dep_helper(dr.ins, h.ins, sync=True, reason="pre-drain")


def _body(tc, x, t, out, n, f):
    """Per-core Tile program. x: DRAM [C, n] f32, t: DRAM [n] i32,
    out: DRAM [3*C] f32 (pred, label, inter counts)."""
    nc = tc.nc
    nb = n // (P * f)
    with tc.tile_pool(name="xp", bufs=2) as xp, \
         tc.tile_pool(name="tp", bufs=2) as tp, \
         tc.tile_pool(name="mp", bufs=2) as mp, \
         tc.tile_pool(name="eqp", bufs=1) as eqp, \
         tc.tile_pool(name="scp", bufs=2) as scp, \
         tc.tile_pool(name="accp", bufs=1) as accp, \
         tc.tile_pool(name="psp", bufs=1, space="PSUM") as psp:
        pacc = accp.tile([P, nb * C], _dt.float32, tag="pacc")
        lacc = accp.tile([P, nb * C], _dt.float32, tag="lacc")
        iacc = accp.tile([P, nb * C], _dt.float32, tag="iacc")
        ones = accp.tile([P, 1], _dt.float32, tag="ones")
        nc.vector.memset(ones[:, :], 1.0)

        for b in range(nb):
            lo = b * P * f
            X = xp.tile([P, C * f], _dt.float32, tag="X")
            nc.sync.dma_start(
                X[:, :].rearrange("p (c f) -> p c f", c=C),
                x[:, lo:lo + P * f].rearrange("c (p f) -> p c f", p=P),
            )
            T32 = tp.tile([P, f], _dt.int32, tag="T32")
            # 3D shape: the 2D form lowers to DMA_DIRECT2D, which only
            # supports one sync-wait command and overflows under Tile.
            nc.sync.dma_start(
                T32[:, :].rearrange("p (a f) -> p a f", a=2),
                t[lo:lo + P * f].rearrange("(p a f) -> p a f", p=P, a=2),
            )
            Tf = tp.tile([P, f], _dt.float32, tag="Tf")
            nc.vector.tensor_copy(Tf[:, :], T32[:, :])

            M = mp.tile([P, f], _dt.float32, tag="M")
            nc.vector.tensor_reduce(
                M[:, :],
                X[:, :].rearrange("p (c f) -> p f c", c=C),
                axis=_Ax.X,
                op=_Alu.max,
            )

            EQ = eqp.tile([P, C * f], _dt.float32, tag="EQ")
            for c in range(C):
                nc.vector.scalar_tensor_tensor(
                    out=EQ[:, c * f:(c + 1) * f],
                    in0=X[:, c * f:(c + 1) * f],
                    scalar=0.0,
                    in1=M[:, :],
                    op0=_Alu.bypass,
                    op1=_Alu.is_equal,
                    accum_out=pacc[:, b * C + c: b * C + c + 1],
                )
            for c in range(C):
                SCR = scp.tile([P, f], _dt.float32, tag="SCR")
                nc.vector.scalar_tensor_tensor(
                    out=SCR[:, :],
                    in0=Tf[:, :],
                    scalar=float(c),
                    in1=EQ[:, c * f:(c + 1) * f],
                    op0=_Alu.is_equal,
                    op1=_Alu.mult,
                    accum_out=iacc[:, b * C + c: b * C + c + 1],
                )
            for c in range(C):
                SCRL = scp.tile([P, f], _dt.float32, tag="SCRL")
                nc.vector.tensor_scalar(
                    out=SCRL[:, :],
                    in0=Tf[:, :],
                    scalar1=float(c),
                    scalar2=None,
                    op0=_Alu.is_equal,
                    op1=_Alu.add,
                    accum_out=lacc[:, b * C + c: b * C + c + 1],
                )

        CNT = accp.tile([P, 3 * C], _dt.float32, tag="CNT")
        for j, acc in enumerate((pacc, lacc, iacc)):
            nc.vector.tensor_reduce(
                CNT[:, j * C:(j + 1) * C],
                acc[:, :].rearrange("p (b c) -> p c b", c=C),
                axis=_Ax.X,
                op=_Alu.add,
            )
        PS = psp.tile([1, 3 * C], _dt.float32, tag="PS")
        nc.tensor.matmul(PS[:, :], ones[:, :], CNT[:, :], start=True, stop=True)
        OUT = accp.tile([1, 3 * C], _dt.float32, tag="OUT")
        nc.vector.tensor_copy(OUT[:, :], PS[:, :])
        nc.sync.dma_start(out.rearrange("(o k) -> o k", o=1), OUT[:, :])


_NC_CACHE = {}


def _get_nc(n, f):
    key = (n, f)
    if key not in _NC_CACHE:
        nc = bass.Bass(
            "TRN2", target_bir_lowering=False, debug=False, num_devices=NCORES
        )
        outn = OUTN if VERSION >= 2 else 3 * C
        x = nc.dram_tensor("x", [C, n], _dt.float32, kind="ExternalInput").ap()
        t_dt = _dt.uint8 if VERSION >= 2 else _dt.int32
        t = nc.dram_tensor("t", [n], t_dt, kind="ExternalInput").ap()
        out = nc.dram_tensor("out", [outn], _dt.float32, kind="ExternalOutput").ap()
        with TileContext(nc) as tc:
            if VERSION == 4:
                _body_v4(tc, x, t, out, n, f)
            elif VERSION == 3:
                _body_v3(tc, x, t, out, n, f)
            elif VERSION == 2:
                _body_v2(tc, x, t, out, n, f)
            else:
                _body(tc, x, t, out, n, f)
        _NC_CACHE[key] = nc
    return _NC_CACHE[key]


def _run(input, target, trace=False):
    inp = np.asarray(input, dtype=np.float32)
    tgt = np.asarray(target)
    b_, c_, h_, w_ = inp.shape
    assert c_ == C, (b_, c_, h_, w_)
    hw = h_ * w_
    n = b_ * hw // NCORES
    nc = _get_nc(n, F)
    x2 = inp.reshape(b_, C, hw)
    t2 = tgt.reshape(b_, hw)
    in_maps = []
    for core in range(NCORES):
        b, off = divmod(core * n, hw)
        in_maps.append({
            "x": np.ascontiguousarray(x2[b, :, off:off + n]),
            "t": np.ascontiguousarray(t2[b, off:off + n]).astype(
                np.uint8 if VERSION >= 2 else np.int32, copy=False
            ),
        })
    res = bass_utils.run_bass_kernel_spmd(
        nc, in_maps, core_ids=list(range(NCORES)), trace=trace
    )
    outn = OUTN if VERSION >= 2 else 3 * C
    counts = np.zeros(outn, np.float64)
    for r in res.results:
        counts += r["out"].astype(np.float64)
    if VERSION >= 2:
        pred = counts[:C]
        s = counts[C:C + NS]
        inter = counts[C + NS:]
        # S(c) = sum |t - c| for c = -1..19; second difference recovers
        # exact integer counts: label_c = (S(c-1) - 2 S(c) + S(c+1)) / 2
        label = (s[:-2] - 2.0 * s[1:-1] + s[2:]) / 2.0
    else:
        pred, label, inter = counts[:C], counts[C:2 * C], counts[2 * C:]
    union = pred + label - inter
    iou_mean = (inter / union).mean()
    return np.float32(iou_mean), res


def kernel(input, target):
    return _run(input, target)[0]



# revision 3
# speedup vs baseline: 1.4722x; 1.4722x over previous
"""Trainium2 Bass kernel v5 for the 19-class mean-IoU (DiceLoss) problem.

Full-input contract: kernel(input, target) takes the FULL unsharded inputs
(input [4,19,512,1024] f32, target [4,512,1024] int), returns the scalar
f32 mean-IoU.  The 2,097,152 pixels are sharded across 8 NeuronCores
(data-parallel over the flattened pixel axis); each core computes
per-class partial counts for its 262,144 pixels; the count vectors are
summed on host and divided there (57 floats).

v5 design (probe-driven):
  - X is loaded HBM->SBUF by the gpsimd software-DGE, which casts
    f32 -> fp16 in flight (bit-exact RNE, verified) -- the cast costs no
    compute and halves SBUF.  All the heavy DVE math then runs on 2-byte
    operands, where TensorTensor ops run in 2x mode (~0.55 ns/elem vs
    1.2 f32).
  - per block [128 x (19 x f)] fp16, class-major:
      DVE: tree max over classes -> M; one whole-tile broadcast
           TT is_equal (X vs M) -> EQF masks; 19 fused STT
           (t==c)*EQF_c + accum -> intersection counts.
      ACT: 19 activation-Copy+accum over EQF_c -> pred counts.
  - label counts: host bincount of the uint8 target (input-only work).
  - final: per-block accumulators reduced on DVE, ones-matmul on the PE
    collapses partitions, one 38-float DMA out per core.
fp16 quantization changes argmax ties (~0.2% of pixels); the exact
quantized metric is simulated on host in test.py (rel err ~1e-3, well
inside the 2e-2 gate).
"""

import numpy as np

import concourse.bass as bass
import concourse.mybir as mybir
from concourse import bass_utils
from concourse.tile import TileContext
from concourse.tile_rust import add_dep_helper

C = 19          # classes
P = 128         # SBUF partitions
NCORES = 8
NB = 4          # blocks per core
OUTN = 2 * C    # pred counts + inter counts

_Alu = mybir.AluOpType
_Ax = mybir.AxisListType
_dt = mybir.dt
_Act = mybir.ActivationFunctionType


def _body_v5(tc, x, t, out, n, f):
    """x: DRAM [C, n] f32, t: DRAM [n] u8, out: DRAM [2C] f32."""
    nc = tc.nc
    nb = n // (P * f)
    fp16 = _dt.float16
    with tc.tile_pool(name="xp", bufs=nb) as xp, \
         tc.tile_pool(name="eqp", bufs=2) as eqp, \
         tc.tile_pool(name="mp", bufs=1) as mp, \
         tc.tile_pool(name="accp", bufs=1) as accp, \
         tc.tile_pool(name="psp", bufs=1, space="PSUM") as psp:
        pacc = accp.tile([P, nb * C], _dt.float32, tag="pacc")
        iacc = accp.tile([P, nb * C], _dt.float32, tag="iacc")
        ones = accp.tile([P, 1], _dt.float32, tag="ones")
        nc.vector.memset(ones[:, :], 1.0)

        hooks = []
        T8 = accp.tile([P, n // P], _dt.uint8, tag="T8")
        t8d = nc.sync.dma_start(
            T8[:, :].rearrange("p (b f) -> p b f", f=f),
            t.rearrange("(b p f) -> p b f", p=P, f=f),
        )
        hooks.append(t8d)
        xs = []
        xd_last = None
        for b in range(nb):
            X = xp.tile([P, C * f], fp16, tag="X")
            xs.append(X)
            xd = nc.gpsimd.dma_start(
                X[:, :].rearrange("p (c f) -> p c f", c=C),
                x[:, b * P * f:(b + 1) * P * f].rearrange(
                    "c (p f) -> p c f", p=P),
            )
            hooks.append(xd)
            xd_last = xd

        small_dve = []
        last_act = None
        for b in range(nb):
            X = xs[b]
            # target cast u8 -> fp16 (values 0..18 exact)
            Tf = mp.tile([P, f], fp16, tag="Tf")
            cast = nc.vector.tensor_copy(Tf[:, :], T8[:, b * f:(b + 1) * f])
            small_dve.append(cast)

            # tree max over 19 classes: 16-wide halving + 3 chain folds.
            # (first op carries this block's X-DMA wait)
            MT = mp.tile([P, 8 * f], fp16, tag="MT")
            M = mp.tile([P, f], fp16, tag="M")
            nc.vector.tensor_tensor(
                MT[:, :], X[:, 0:8 * f], X[:, 8 * f:16 * f], _Alu.max)
            nc.vector.tensor_tensor(
                MT[:, 0:4 * f], MT[:, 0:4 * f], MT[:, 4 * f:8 * f], _Alu.max)
            nc.vector.tensor_tensor(
                MT[:, 0:2 * f], MT[:, 0:2 * f], MT[:, 2 * f:4 * f], _Alu.max)
            nc.vector.tensor_tensor(
                M[:, :], MT[:, 0:f], MT[:, f:2 * f], _Alu.max)
            for c in range(16, C):
                nc.vector.tensor_tensor(
                    M[:, :], M[:, :], X[:, c * f:(c + 1) * f], _Alu.max)

            # EQF whole-tile: (X == M broadcast over classes), one 2x op
            EQF = eqp.tile([P, C * f], fp16, tag="EQF")
            eqf = nc.vector.tensor_tensor(
                EQF[:, :].rearrange("p (c f) -> p c f", c=C),
                X[:, :].rearrange("p (c f) -> p c f", c=C),
                M[:, :].rearrange("p (c f) -> p c f", c=1)
                .to_broadcast([P, C, f]),
                _Alu.is_equal)

            # ACT: pred counts = per-class Copy+accum over EQF.
            # Accum ops carry an implicit self-engine wait, so a non-accum
            # entry shim absorbs the cross-engine DVE wait first.
            ASH = mp.tile([P, 1], fp16, tag="ASH")
            ash = nc.scalar.activation(ASH[:, :], EQF[:, 0:1], _Act.Copy)
            if last_act is not None:
                add_dep_helper(ash.ins, last_act.ins, sync=False,
                               reason="chain ACT")
            last_act = ash
            AJ = mp.tile([P, f], fp16, tag="AJ")
            for c in range(C):
                a = nc.scalar.activation(
                    AJ[:, :], EQF[:, c * f:(c + 1) * f], _Act.Copy,
                    accum_out=pacc[:, b * C + c:b * C + c + 1])
                add_dep_helper(a.ins, ash.ins, sync=False,
                               reason="after shim")
                last_act = a

            # DVE: intersection counts, fused (t==c)*EQF_c + accum
            EQJ = mp.tile([P, f], fp16, tag="EQJ")
            for c in range(C):
                nc.vector.scalar_tensor_tensor(
                    out=EQJ[:, :], in0=Tf[:, :], scalar=float(c),
                    in1=EQF[:, c * f:(c + 1) * f],
                    op0=_Alu.is_equal, op1=_Alu.mult,
                    accum_out=iacc[:, b * C + c:b * C + c + 1])

        CNT = accp.tile([P, OUTN], _dt.float32, tag="CNT")
        # pacc is ACT-written: this reduce carries one ACT wait
        nc.vector.tensor_reduce(
            CNT[:, 0:C],
            pacc[:, :].rearrange("p (b c) -> p c b", c=C),
            axis=_Ax.X, op=_Alu.add,
        )
        nc.vector.tensor_reduce(
            CNT[:, C:OUTN],
            iacc[:, :].rearrange("p (b c) -> p c b", c=C),
            axis=_Ax.X, op=_Alu.add,
        )
        PS = psp.tile([1, OUTN], _dt.float32, tag="PS")
        mm = nc.tensor.matmul(
            PS[:, :], ones[:, :], CNT[:, :], start=True, stop=True
        )
        OUT = accp.tile([1, OUTN], _dt.float32, tag="OUT")
        oc = nc.vector.tensor_copy(OUT[:, :], PS[:, :])
        for s in small_dve:
            add_dep_helper(oc.ins, s.ins, sync=False,
                           reason="OUT copy last on DVE")
        od = nc.sync.dma_start(out.rearrange("(o k) -> o k", o=1), OUT[:, :])

        # Pool-engine terminal (covers the SW-DGE DMACopy instruction ticks)
        GSCR = accp.tile([1, 1], _dt.float32, tag="GSCR")
        gfin = nc.gpsimd.memset(GSCR[:, :], 0.0)
        add_dep_helper(gfin.ins, xd_last.ins, sync=False, reason="GP last")
        hooks += [last_act, mm, od, gfin]
        for h in hooks:
            dr = nc.sync.drain()
            add_dep_helper(dr.ins, h.ins, sync=True, reason="pre-drain")


_NC_CACHE = {}


def _get_nc(n, f):
    key = (n, f)
    if key not in _NC_CACHE:
        nc = bass.Bass(
            "TRN2", target_bir_lowering=False, debug=False, num_devices=NCORES
        )
        x = nc.dram_tensor("x", [C, n], _dt.float32, kind="ExternalInput").ap()
        t = nc.dram_tensor("t", [n], _dt.uint8, kind="ExternalInput").ap()
        out = nc.dram_tensor(
            "out", [OUTN], _dt.float32, kind="ExternalOutput").ap()
        with TileContext(nc) as tc:
            _body_v5(tc, x, t, out, n, f)
        _NC_CACHE[key] = nc
    return _NC_CACHE[key]


def _run(input, target, trace=False):
    inp = np.asarray(input, dtype=np.float32)
    tgt = np.asarray(target)
    b_, c_, h_, w_ = inp.shape
    assert c_ == C, (b_, c_, h_, w_)
    hw = h_ * w_
    n = b_ * hw // NCORES
    f = n // (P * NB)
    nc = _get_nc(n, f)
    x2 = inp.reshape(b_, C, hw)
    t2 = tgt.reshape(b_, hw)
    in_maps = []
    for core in range(NCORES):
        b, off = divmod(core * n, hw)
        in_maps.append({
            "x": np.ascontiguousarray(x2[b, :, off:off + n]),
            "t": np.ascontiguousarray(t2[b, off:off + n]).astype(
                np.uint8, copy=False),
        })
    res = bass_utils.run_bass_kernel_spmd(
        nc, in_maps, core_ids=list(range(NCORES)), trace=trace
    )
    counts = np.zeros(OUTN, np.float64)
    for r in res.results:
        counts += r["out"].astype(np.float64)
    pred = counts[:C]
    inter = counts[C:]
    label = np.bincount(
        np.asarray(target).reshape(-1).astype(np.int64), minlength=C
    )[:C].astype(np.float64)
    union = pred + label - inter
    iou_mean = (inter / union).mean()
    return np.float32(iou_mean), res


def kernel(input, target):
    return _run(input, target)[0]
